# revision 37
# baseline (speedup 1.0000x reference)
"""Trainium2 Bass kernel for nn_CpGPredictor (pairwise-token logistic head).

Math: out[b, s] = emb[x[b,s]] . w_prev + emb[x[b,s+1]] . w_curr + bias
With VOCAB=5 the embedding+linear collapses to two 5-entry scalar tables
    p[v] = emb[v] . w_prev,   c[v] = emb[v] . w_curr  (+ bias)
interpolated exactly by quartics:
    p(a) = sgp*Sq(sp*Sq(a+alp) + bp) + rp*a + cp
    c(b) = sgc*Sq(sc*Sq(b+alc2) + bc2) + rc*b + cc

v2 structure: full-width [128,1024] ops; the 5-term combine runs as a
scalar_tensor_tensor (STT) chain - (in0*s)+in1 fused per op:
    R   = rp*a + K''              [DVE TS, u8 even-offset]
    R2  = rc*WB + R               [STT halves on DVE/Pool]
    U   = sgp*YA + R2             [STT halves]
    OUT = sgc*YB + U              [STT halves]
with WB = b+alc2 (Pool, odd-offset u8 ok there), SB = WB^2 (DVE TT),
GA/YA/YB on ACT.  Tails are column-split DVE||Pool; outputs DMA'd per
half from the two HWDGE rings (sync + scalar).

Device layout (pure data parallel over batch, 8 NeuronCores):
  - tokens shipped as uint8; each core gets [16, 8193] (last col = pad)
  - each row split into 8 overlapping chunks of 1025 -> 128 partitions
  - input lands as two column-halves of one X[128,1025] buffer via both
    HWDGE rings concurrently (~1.8us vs 2.8us single-DMA)
  - gpsimd same-engine RAW deps self-synced via psem (Q7s are async)

Self-contained: hardcodes B=128, S=8192, VOCAB=5, 8 cores.
"""

import os
import sys

import numpy as np

for _p in ("/opt/trn_rl_repo", "/root/.axon_site/_ro/trn_rl_repo"):
    if _p not in sys.path and os.path.isdir(_p):
        sys.path.append(_p)

B = 128
S = 8192
VOCAB = 5
EMBED = 128
N_CORES = 8
ROWS = B // N_CORES          # 16 rows per core
CHUNKS = 8                   # chunks per row -> 128 partitions
CHUNK = S // CHUNKS          # 1024 output elements per partition
SPAD = S + 1                 # padded row length
XW = CHUNK + 1               # 1025 tokens per partition
HW0 = 513                    # half0 = cols [0, 513)
TL = 512                     # left/right column split for the tail ops

_STATE = {}


def _params(emb_table, lin_w, lin_b):
    """Host-side f64: fold emb+linear+bias into the kernel immediates."""
    emb = np.asarray(emb_table, np.float64)
    lw = np.asarray(lin_w, np.float64).reshape(-1)
    bias = float(np.asarray(lin_b, np.float64).reshape(-1)[0])
    p = emb @ lw[:EMBED]
    c = emb @ lw[EMBED:] + bias

    t = np.arange(VOCAB, dtype=np.float64)
    V = np.vander(t, VOCAB, increasing=True)

    def quartic(vals):
        a = np.linalg.solve(V, vals)
        if abs(a[4]) < 1e-7:
            vals = vals + 1e-6 * np.array([1.0, -4.0, 6.0, -4.0, 1.0])
            a = np.linalg.solve(V, vals)
        return a

    ap = quartic(p)
    alp = ap[3] / (4 * ap[4])
    c0 = ap[2] / (2 * ap[4]) - 2 * alp * alp
    qp = c0 - alp * alp
    rp = ap[1] - 4 * ap[4] * alp * c0
    cp = ap[0] - ap[4] * c0 * c0
    sgp = 1.0 if ap[4] > 0 else -1.0
    sp = np.sqrt(abs(ap[4]))
    bp = qp * sp

    ac = quartic(c)
    alc = ac[3] / (2 * ac[4])
    qc = (ac[2] / ac[4] - alc * alc) / 2
    rc = ac[1] - 2 * ac[4] * alc * qc
    cc = ac[0] - ac[4] * qc * qc
    sgc = 1.0 if ac[4] > 0 else -1.0
    sc = np.sqrt(abs(ac[4]))
    bc = qc * sc

    alc2 = alc / 2
    bc2 = bc - sc * alc2 * alc2    # inner as (b+alc/2)^2; fold -alc^2/4 here
    # R = rp*a + K2 with K2 folding the constants and rc*alc2 (rc rides WB)
    K2 = cp + cc - rc * alc2
    f = float
    return dict(alp=f(alp), sp=f(sp), bp=f(bp), sgp=f(sgp), rp=f(rp),
                alc2=f(alc2), sc=f(sc), bc2=f(bc2), sgc=f(sgc), rc=f(rc),
                K2=f(K2))


def _build_nc(P):
    import concourse.bass as bass
    import concourse.mybir as mybir
    from concourse.ap import AP

    f32 = mybir.dt.float32
    f16 = mybir.dt.float16
    u8 = mybir.dt.uint8
    MUL = mybir.AluOpType.mult
    ADD = mybir.AluOpType.add
    SUB = mybir.AluOpType.subtract
    SQ = mybir.ActivationFunctionType.Square
    OPP = ADD if P["sgp"] > 0 else SUB   # U = R2 +- YA
    OPC = ADD if P["sgc"] > 0 else SUB   # OUT = U +- YB

    nc = bass.Bass()
    x_ext = nc.dram_tensor("xin", [ROWS, SPAD], u8, kind="ExternalInput")
    y_ext = nc.dram_tensor("yout", [ROWS, S], f16, kind="ExternalOutput")
    y_dst = y_ext[:, :].rearrange("r (c j) -> (r c) j", j=CHUNK)

    X = nc.alloc_sbuf_tensor("X", [128, XW], u8)

    WB = nc.alloc_sbuf_tensor("WB", [128, CHUNK], f16)
    SB = nc.alloc_sbuf_tensor("SB", [128, CHUNK], f16)
    GA = nc.alloc_sbuf_tensor("GA", [128, CHUNK], f16)
    YA = nc.alloc_sbuf_tensor("YA", [128, CHUNK], f16)
    YB = nc.alloc_sbuf_tensor("YB", [128, CHUNK], f16)
    R = nc.alloc_sbuf_tensor("R", [128, CHUNK], f16)
    R2 = nc.alloc_sbuf_tensor("R2", [128, CHUNK], f16)
    U = nc.alloc_sbuf_tensor("U", [128, CHUNK], f16)
    OUT = nc.alloc_sbuf_tensor("OUT", [128, CHUNK], f16)
    BIAS = nc.alloc_sbuf_tensor("BIAS", [128, 3], f32)
    DUMMY = nc.alloc_sbuf_tensor("DUMMY", [128, 1], f16)

    dsA = nc.alloc_semaphore("dsA")
    dsB = nc.alloc_semaphore("dsB")
    vsem = nc.alloc_semaphore("vsem")
    psem = nc.alloc_semaphore("psem")
    asem = nc.alloc_semaphore("asem")
    osem = nc.alloc_semaphore("osem")

    def L(t):
        return t[:, 0:TL]

    def Rt(t):
        return t[:, TL:CHUNK]

    # pre-Block: half0 input DMA on the SP HWDGE ring
    srcA = AP(x_ext, 0, [[SPAD, ROWS], [CHUNK, CHUNKS], [1, HW0]])
    nc.sync.dma_start(X[:, 0:HW0], srcA).then_inc(dsA, 16)

    with nc.Block(no_gpsimd_drain=True) as block:

        @block.sync
        def _(sync):
            # left output half
            sync.wait_ge(vsem, 8)
            sync.dma_start(y_dst[:, 0:TL], L(OUT)).then_inc(osem, 16)

        @block.scalar
        def _(scalar):
            # half1 input DMA on the ACT HWDGE ring (cols [513,1025))
            srcB = AP(x_ext, HW0, [[SPAD, ROWS], [CHUNK, CHUNKS], [1, TL]])
            scalar.dma_start(X[:, HW0:XW], srcB).then_inc(dsB, 16)
            # table preload for Square during the DMA flight
            const0 = nc.const_aps.tensor(0.0, (128, 1), f32)
            scalar.activation(out=DUMMY[:], in_=const0, func=SQ,
                              bias=0.0, scale=1.0)
            scalar.wait_ge(vsem, 3)
            scalar.wait_ge(dsA, 16)
            scalar.wait_ge(dsB, 16)
            scalar.activation(out=GA[:, :], in_=X[:, 0:CHUNK], func=SQ,
                              bias=BIAS[:, 0:1],
                              scale=1.0).then_inc(asem, 1)
            scalar.wait_ge(asem, 1)
            scalar.activation(out=YA[:, :], in_=GA[:, :], func=SQ,
                              bias=BIAS[:, 1:2],
                              scale=P["sp"]).then_inc(asem, 1)
            scalar.wait_ge(vsem, 5)   # SB_left
            scalar.wait_ge(psem, 3)   # SB_right
            scalar.activation(out=YB[:, :], in_=SB[:, :], func=SQ,
                              bias=BIAS[:, 2:3],
                              scale=P["sc"]).then_inc(asem, 1)
            # right output half
            scalar.wait_ge(vsem, 9)
            scalar.dma_start(y_dst[:, TL:CHUNK], Rt(OUT)).then_inc(osem, 16)

        @block.gpsimd
        def _(gpsimd):
            # WB = b + alc2, two halves (odd-offset u8 is fine on Pool)
            gpsimd.wait_ge(dsA, 16)
            gpsimd.tensor_scalar(out=L(WB), in0=X[:, 1:TL + 1],
                                 scalar1=1.0, scalar2=P["alc2"],
                                 op0=MUL, op1=ADD).then_inc(psem, 1)  # 1
            gpsimd.wait_ge(dsB, 16)
            gpsimd.tensor_scalar(out=Rt(WB), in0=X[:, TL + 1:XW],
                                 scalar1=1.0, scalar2=P["alc2"],
                                 op0=MUL, op1=ADD).then_inc(psem, 1)  # 2
            # SB_right here so DVE can run R2 in parallel
            gpsimd.wait_ge(psem, 2)
            gpsimd.tensor_tensor(out=Rt(SB), in0=Rt(WB),
                                 in1=Rt(WB), op=MUL).then_inc(psem, 1)  # 3

        @block.vector
        def _(vector):
            vector.memset(BIAS[:, 0:1], P["alp"]).then_inc(vsem, 1)
            vector.memset(BIAS[:, 1:2], P["bp"]).then_inc(vsem, 1)
            vector.memset(BIAS[:, 2:3], P["bc2"]).then_inc(vsem, 1)
            vector.wait_ge(dsA, 16)
            vector.wait_ge(dsB, 16)
            # R = rp*a + K2 (even-offset u8 fine on DVE)
            vector.tensor_scalar(out=R[:, :], in0=X[:, 0:CHUNK],
                                 scalar1=P["rp"], scalar2=P["K2"],
                                 op0=MUL, op1=ADD).then_inc(vsem, 1)  # 4
            # SB_left behind WB_left
            vector.wait_ge(psem, 1)
            vector.tensor_tensor(out=L(SB), in0=L(WB),
                                 in1=L(WB), op=MUL).then_inc(vsem, 1)  # 5
            # R2 = rc*WB + R as soon as WB_right lands (before SB_right:
            # it gates the U chain; SB_right only gates YB via ACT queue)
            vector.wait_ge(psem, 2)
            vector.wait_ge(vsem, 4)   # self: R
            vector.scalar_tensor_tensor(out=R2[:, :], in0=WB[:, :],
                                        scalar=P["rc"], in1=R[:, :],
                                        op0=MUL, op1=ADD).then_inc(vsem, 1)  # 6
            # U = sgp*YA + R2 (STT full; TT-subtract has no 2x uop)
            vector.wait_ge(vsem, 6)   # self: R2
            vector.wait_ge(asem, 2)   # YA
            vector.scalar_tensor_tensor(out=U[:, :], in0=YA[:, :],
                                        scalar=P["sgp"], in1=R2[:, :],
                                        op0=MUL, op1=ADD).then_inc(vsem, 1)  # 7
            # OUT halves = U +- YB
            vector.wait_ge(vsem, 7)   # self: U
            vector.wait_ge(asem, 3)   # YB
            vector.tensor_tensor(out=L(OUT), in0=L(U), in1=L(YB),
                                 op=OPC).then_inc(vsem, 1)  # 8
            vector.wait_ge(vsem, 8)
            vector.tensor_tensor(out=Rt(OUT), in0=Rt(U), in1=Rt(YB),
                                 op=OPC).then_inc(vsem, 1)  # 9

    return nc


def _get_nc(P):
    key = tuple(sorted(P.items()))
    if _STATE.get("key") != key:
        _STATE["nc"] = _build_nc(P)
        _STATE["key"] = key
    return _STATE["nc"]


def _run(x, emb_table, lin_w, lin_b, trace=False):
    from concourse.bass_utils import run_bass_kernel_spmd

    P = _params(emb_table, lin_w, lin_b)

    xq = np.asarray(x)
    assert xq.shape == (B, S), xq.shape
    xpad = np.zeros((B, SPAD), np.uint8)
    xpad[:, :S] = xq.astype(np.uint8)

    in_maps = [
        {"xin": np.ascontiguousarray(xpad[ROWS * i:ROWS * (i + 1)])}
        for i in range(N_CORES)
    ]
    nc = _get_nc(P)
    res = run_bass_kernel_spmd(nc, in_maps, list(range(N_CORES)), trace=trace)
    y = np.concatenate([res.results[i]["yout"] for i in range(N_CORES)], axis=0)
    return np.ascontiguousarray(y[:, :S - 1]).astype(np.float32), res


def kernel(x, emb_table, lin_w, lin_b):
    y, _ = _run(x, emb_table, lin_w, lin_b, trace=False)
    return y
